# revision 29
# baseline (speedup 1.0000x reference)
"""Trainium2 Bass kernel for nn_DeepFakeDetectionModel (dense MLP).

Model: stem LN(26)->Linear(26->1024)->ReLU, 8x [LN(1024)->Linear(1024->1024)->ReLU],
head LN(1024)->Linear(1024->1)->sigmoid.  B=65536 sharded over 8 cores (data parallel,
8192 rows/core), parameters replicated.

Per-core design (batch-major pipeline, fp16 matmuls):
- Host folds each LN's affine (gamma/beta) into the following Linear:
    W' = W * gamma[None, :],  b' = b + W @ beta
  so the device only computes plain normalization (x - mean) * rsqrt(var + eps).
- Activations live as [128 batch, 1024 feat] tiles in SBUF.  The normalized tile is
  transposed by the DMA xbar (SBUF->SBUF, fp16, one instruction per tile) into the
  chunk-major layout zt[p, c, b] = z[b, 128c + p], keeping the TensorEngine free for
  matmuls only: per tile-layer 8 LDWEIGHTS (zt chunk stationary) + 16 matmuls
  (weights [128, 512] moving) accumulate [128 batch, 1024 feat] fp32 in PSUM.
- Epilogue: bias add (DVE) -> ReLU with sum accumulator (ACT) -> Square with
  sum-of-squares accumulator (ACT) -> small stats ops -> fused normalize (DVE).
- Head is a single DVE multiply-reduce against the broadcast last-layer weights
  (no transpose, no matmul); one sigmoid pass at the end.
- All block weights stay resident in SBUF as fp16.
"""

import os
from contextlib import ExitStack

import numpy as np

import concourse.bass as bass
import concourse.bacc as bacc
import concourse.mybir as mybir
import concourse.tile as tile
from concourse.bass_utils import run_bass_kernel_spmd

EPS = 1e-5
P = 128
H = 1024
F_IN = 26
L = 8
KC = H // P  # k-chunks per 1024 contraction
NJ = H // 512  # 512-wide output slices per matmul group
N_CORES = 8
B = 65536
ROWS = B // N_CORES

DT16 = mybir.dt.float16
F32 = mybir.dt.float32
NP16 = np.float16

LAST_RESULTS = None


def _bcast_ap(ap, parts):
    """Prepend a stride-0 partition dim of size `parts` to a DRAM AP."""
    return bass.AP(tensor=ap.tensor, offset=ap.offset, ap=[[0, parts]] + list(ap.ap))


def _emit(ctx, tc, rows, head_bias, x_ap, stw_ap, wt_ap, bb_ap, lw_ap, out_ap):
    nc = tc.nc
    ntiles = rows // P
    G = 5  # tiles interleaved per group
    Sqrt = mybir.ActivationFunctionType.Sqrt
    Relu = mybir.ActivationFunctionType.Relu
    Sigmoid = mybir.ActivationFunctionType.Sigmoid
    Square = mybir.ActivationFunctionType.Square
    sub = mybir.AluOpType.subtract
    mult = mybir.AluOpType.mult
    add = mybir.AluOpType.add

    const = ctx.enter_context(tc.tile_pool(name="const", bufs=1))

    eps_t = const.tile([P, 1], F32)
    nc.vector.memset(eps_t, EPS)

    zero_t = const.tile([P, 1], F32)
    nc.vector.memset(zero_t, 0.0)

    hb_t = const.tile([P, 1], F32)
    nc.vector.memset(hb_t, head_bias)

    # shared write-only scratch for accumulator-producing ops (never read;
    # same-engine writes serialize in order)
    junk_sq = const.tile([P, H], DT16, name="junk_sq")   # scalar engine
    junk_dve = const.tile([P, H], DT16, name="junk_dve")  # vector engine

    tmppool = ctx.enter_context(tc.tile_pool(name="tmp", bufs=4))       # hb (psum+bias)
    hpool = ctx.enter_context(tc.tile_pool(name="hp", bufs=2 * G))      # z state
    szpool = ctx.enter_context(tc.tile_pool(name="sz", bufs=G))         # next-group stem z
    ztpool = ctx.enter_context(tc.tile_pool(name="ztp", bufs=4))
    zxpool = ctx.enter_context(tc.tile_pool(name="zxp", bufs=3))
    stpool = ctx.enter_context(tc.tile_pool(name="stp", bufs=2 * G))
    xpool = ctx.enter_context(tc.tile_pool(name="xg", bufs=3))
    ppool = ctx.enter_context(tc.tile_pool(name="pp", bufs=4, space="PSUM"))

    x_view = x_ap.rearrange("(t p) f -> p t f", p=P)

    def load_x(tiles):
        x_g = xpool.tile([P, len(tiles), F_IN], F32, tag="xg", name="x_g")
        nc.sync.dma_start(out=x_g, in_=x_view[:, tiles[0]:tiles[0] + len(tiles), :])
        return x_g

    groups = [list(range(g0, min(g0 + G, ntiles))) for g0 in range(0, ntiles, G)]

    # Queue split: the Sync (SP HWDGE) ring carries only the small, latency-
    # critical transfers (x tiles, stem weights, activation transposes); the
    # bulk weight/bias preload goes on the GpSimd SWDGE ring, which nothing
    # latency-critical uses during the preload window (the first group's
    # small stats ops run on DVE instead, see block_norm smalls).  Group 0's x comes in two halves so the
    # first stems never wait on the later tiles' data.
    g0a, g0b = groups[0][:3], groups[0][3:]
    x_ga = load_x(g0a)
    x_gb = load_x(g0b)

    # stem weights [128, 1024] fp16 (row 26 = stem bias, rows 27.. zero)
    stw_t = const.tile([P, H], DT16)
    nc.sync.dma_start(out=stw_t, in_=stw_ap)

    # block weights, resident: per layer [128, 8, 1024] fp16 (chunk c = rows 128c..)
    wt_t = [const.tile([P, KC, H], DT16, name=f"wt{l}") for l in range(L)]
    wt_view = wt_ap.rearrange("l (c p) o -> l p c o", p=P)
    nc.gpsimd.dma_start(out=wt_t[0], in_=wt_view[0])

    # biases, broadcast across all 128 partitions: [128, 9, 1024] fp16
    bb_t = const.tile([P, L + 1, H], DT16)
    nc.gpsimd.dma_start(out=bb_t, in_=_bcast_ap(bb_ap, P))

    for l in range(1, L):
        nc.gpsimd.dma_start(out=wt_t[l], in_=wt_view[l])

    # head weights broadcast across partitions: [128, 1024] fp16
    lw_t = const.tile([P, H], DT16)
    nc.gpsimd.dma_start(out=lw_t, in_=_bcast_ap(lw_ap, P))

    # logits accumulated on-chip; one sigmoid pass at the end
    o_all = const.tile([P, ntiles, 1], F32, name="o_all")

    inv_h = 1.0 / H

    def block_mm(z, l, warm=False):
        """DMA-xbar transpose z -> zt chunks, then 16 matmuls into a psum tile."""
        zt = ztpool.tile([P, KC, P], DT16, tag="zt", name="zt")
        nc.sync.dma_start(out=zt, in_=z, transpose=True)
        p_t = ppool.tile([P, H], F32, tag="p", name="p_t")
        for c in range(KC):
            for j in range(NJ):
                nc.tensor.matmul(p_t[:, j * 512:(j + 1) * 512],
                                 lhsT=zt[:, c, :],
                                 rhs=wt_t[l][:, c, j * 512:(j + 1) * 512],
                                 start=(c == 0), stop=(c == KC - 1))
        return p_t

    def block_norm(p_t, l, pool, biased=False, smalls=None):
        """psum (+ bias) -> relu(+sum) -> square(+sumsq) -> normalize.

        Returns z = (h - m) * rsqrt(var + eps) as fp16 [P, H] (in the h tile).
        `biased=True` means the bias is already in the psum (stem), so the
        relu reads PSUM directly.
        """
        h_t = pool.tile([P, H], DT16, tag="h", name="h_t")
        s0 = stpool.tile([P, 1], F32, tag="s0", name="s0")
        if biased:
            nc.scalar.activation(out=h_t, in_=p_t, func=Relu, bias=zero_t,
                                 accum_out=s0)
        else:
            hb = tmppool.tile([P, H], DT16, tag="tmp", name="hb")
            nc.vector.tensor_tensor(hb, p_t, bb_t[:, l, :], add)
            nc.scalar.activation(out=h_t, in_=hb, func=Relu, bias=zero_t,
                                 accum_out=s0)
        q_t = stpool.tile([P, 1], F32, tag="q", name="q_t")
        nc.scalar.activation(out=junk_sq, in_=h_t, func=Square, bias=zero_t,
                             accum_out=q_t)
        # tiny [P,1] stats ops run on the otherwise-idle GpSimd engine to keep
        # the DVE FIFO clear for the PSUM-freeing bias adds (DVE during the
        # startup window while gpsimd drains the weight preload)
        eng = smalls if smalls is not None else nc.gpsimd
        mn = stpool.tile([P, 1], F32, tag="mn", name="mn")
        eng.tensor_scalar(mn, s0, 0.0, -inv_h, add, mult)  # -mean
        msq = stpool.tile([P, 1], F32, tag="msq", name="msq")
        eng.tensor_tensor(msq, mn, mn, mult)               # mean^2
        v_t = stpool.tile([P, 1], F32, tag="v", name="v_t")
        eng.tensor_scalar(v_t, q_t, inv_h, msq, mult, sub)  # var
        sd = stpool.tile([P, 1], F32, tag="sd", name="sd")
        nc.scalar.activation(out=sd, in_=v_t, func=Sqrt, bias=eps_t, scale=1.0)
        rv = stpool.tile([P, 1], F32, tag="rv", name="rv")
        nc.vector.reciprocal(out=rv, in_=sd)
        nc.vector.tensor_scalar(h_t, h_t, mn, rv, add, mult)
        return h_t

    def stem_a(i, x_g, gi):
        """x tile -> LN -> padded zx tile (col 26 = constant 1 for the bias row)."""
        x_t = x_g[:, i - gi, :]
        stats = stpool.tile([P, 1, 6], F32, tag="stats", name="stats")
        nc.vector.bn_stats(out=stats[:, 0, :], in_=x_t)
        mv = stpool.tile([P, 2], F32, tag="mv", name="mv")
        nc.vector.bn_aggr(out=mv, in_=stats)
        sd = stpool.tile([P, 1], F32, tag="sd", name="sd")
        nc.scalar.activation(out=sd, in_=mv[:, 1:2], func=Sqrt, bias=eps_t, scale=1.0)
        rv = stpool.tile([P, 1], F32, tag="rv", name="rv")
        nc.vector.reciprocal(out=rv, in_=sd)
        zx = zxpool.tile([P, P], DT16, tag="zx", name="zx")
        nc.vector.memset(zx[:, F_IN + 1:], 0.0)
        nc.vector.memset(zx[:, F_IN:F_IN + 1], 1.0)  # feeds the stem bias row
        nc.vector.tensor_scalar(zx[:, 0:F_IN], x_t, mv[:, 0:1], rv, sub, mult)
        return zx

    def stem_b(zx):
        """xbar transpose of the padded stem input (issued a slot after stem_a
        so it never stalls the Sync DMA FIFO waiting on fresh DVE work)."""
        zxt = zxpool.tile([P, P], DT16, tag="zxt", name="zxt")
        nc.sync.dma_start(out=zxt, in_=zx, transpose=True)
        return zxt

    def stem_c(zxt, smalls=None):
        """stem matmul (bias included via the constant-1 row) -> normalized z."""
        p_t = ppool.tile([P, H], F32, tag="p", name="sp_t")
        for j in range(NJ):
            nc.tensor.matmul(p_t[:, j * 512:(j + 1) * 512], lhsT=zxt,
                             rhs=stw_t[:, j * 512:(j + 1) * 512],
                             start=True, stop=True)
        return block_norm(p_t, 0, szpool, biased=True, smalls=smalls)

    def head(i, z):
        """logit[i] = sum(z * last_w) along features, via one DVE mul-reduce."""
        nc.vector.affine_mul_reduce(out=junk_dve, accum_out=o_all[:, i, :],
                                    in0=z, in1=lw_t, scale=1.0, bias=zero_t)

    # Next-group stems are split into three phases spread over distinct tile
    # slots in layers L-3..L-1 (zx build -> transpose issue -> matmul) so no
    # FIFO (Sync DMA, TensorE, DVE) ever head-of-line-blocks on fresh
    # upstream work, and the extra DVE/ACT load never bursts.
    stem_actions = {}
    for j in range(G):
        stem_actions.setdefault(2 * j, []).append(("a", j))
        stem_actions.setdefault(2 * j + 1, []).append(("b", j))
        stem_actions.setdefault(2 * j + 3, []).append(("c", j))

    zs = {}
    st_state = {}
    for i in g0a:
        st_state[i] = stem_a(i, x_ga, g0a[0])
    for i in g0a:
        st_state[i] = stem_b(st_state[i])
    for i in g0b:
        st_state[i] = stem_a(i, x_gb, g0b[0])
    for i in g0b:
        st_state[i] = stem_b(st_state[i])
    for i in groups[0]:
        zs[i] = stem_c(st_state.pop(i), smalls=nc.vector)
    for gi, tiles in enumerate(groups):
        nxt = groups[gi + 1] if gi + 1 < len(groups) else []
        smalls = nc.vector if gi == 0 else None
        warm = gi <= 1
        for l in range(1, L + 1):
            if l == L - 3 and nxt:
                x_g = load_x(nxt)
            for k, i in enumerate(tiles):
                p_t = block_mm(zs[i], l - 1, warm=warm)
                zs[i] = block_norm(p_t, l, hpool, smalls=smalls)
                if l == L:
                    head(i, zs.pop(i))
                elif nxt and l >= L - 3:
                    slot = (l - (L - 3)) * len(tiles) + k
                    for phase, j in stem_actions.get(slot, []):
                        if j >= len(nxt):
                            continue
                        if phase == "a":
                            st_state[j] = stem_a(nxt[j], x_g, nxt[0])
                        elif phase == "b":
                            st_state[j] = stem_b(st_state[j])
                        else:
                            zs[nxt[j]] = stem_c(st_state.pop(j), smalls=smalls)


    # single sigmoid pass over all logits, then store
    nc.scalar.activation(out=o_all[:, :, 0], in_=o_all[:, :, 0], func=Sigmoid,
                         bias=hb_t, scale=1.0)
    nc.sync.dma_start(out=out_ap.rearrange("(t p) o -> p t o", p=P), in_=o_all)


def build_program(rows, head_bias):
    nc = bacc.Bacc("TRN2", target_bir_lowering=False, debug=False,
                   enable_asserts=False)
    x_ap = nc.dram_tensor("x", [rows, F_IN], F32, kind="ExternalInput").ap()
    stw_ap = nc.dram_tensor("stw", [P, H], DT16, kind="ExternalInput").ap()
    wt_ap = nc.dram_tensor("wt", [L, H, H], DT16, kind="ExternalInput").ap()
    bb_ap = nc.dram_tensor("bb", [L + 1, H], DT16, kind="ExternalInput").ap()
    lw_ap = nc.dram_tensor("lw", [H], DT16, kind="ExternalInput").ap()
    out_ap = nc.dram_tensor("out", [rows, 1], F32, kind="ExternalOutput").ap()
    with tile.TileContext(nc) as tc:
        with ExitStack() as ctx:
            _emit(ctx, tc, rows, head_bias,
                  x_ap, stw_ap, wt_ap, bb_ap, lw_ap, out_ap)
    nc.compile()
    return nc


def preprocess(inputs):
    """Fold LN affines into the following linears; build device-layout arrays."""
    f8 = np.float64
    st_w = np.asarray(inputs["st_w"], f8)
    st_g = np.asarray(inputs["st_gamma"], f8)
    st_be = np.asarray(inputs["st_beta"], f8)
    st_b = np.asarray(inputs["st_b"], f8)
    blk_w = np.asarray(inputs["blk_w"], f8)
    blk_g = np.asarray(inputs["blk_gamma"], f8)
    blk_be = np.asarray(inputs["blk_beta"], f8)
    blk_b = np.asarray(inputs["blk_b"], f8)
    last_w = np.asarray(inputs["last_w"], f8)
    last_g = np.asarray(inputs["last_gamma"], f8)
    last_be = np.asarray(inputs["last_beta"], f8)
    last_b = np.asarray(inputs["last_b"], f8)

    st_wp = st_w * st_g[None, :]
    st_bp = st_b + st_w @ st_be
    blk_wp = blk_w * blk_g[:, None, :]
    blk_bp = blk_b + np.einsum("lhk,lk->lh", blk_w, blk_be)
    last_wp = last_w * last_g[None, :]
    head_bias = float(last_b[0] + last_w[0] @ last_be)

    stw = np.zeros((P, H), NP16)
    stw[:F_IN] = np.ascontiguousarray(st_wp.T).astype(NP16)          # [128, 1024]
    stw[F_IN] = st_bp.astype(NP16)  # stem bias row (paired with constant-1 input col)
    wt = np.ascontiguousarray(blk_wp.transpose(0, 2, 1)).astype(NP16)  # [8, fin, fout]
    bb = np.concatenate([st_bp[None, :], blk_bp], axis=0).astype(NP16)  # [9, 1024]
    lw = np.ascontiguousarray(last_wp[0]).astype(NP16)               # [1024]
    return stw, wt, bb, lw, head_bias


def kernel(**inputs):
    global LAST_RESULTS
    x = np.ascontiguousarray(np.asarray(inputs["x"], dtype=np.float32))
    assert x.shape == (B, F_IN)
    stw, wt, bb, lw, head_bias = preprocess(inputs)

    nc = build_program(ROWS, head_bias)
    in_maps = []
    for c in range(N_CORES):
        in_maps.append({
            "x": np.ascontiguousarray(x[c * ROWS:(c + 1) * ROWS]),
            "stw": stw, "wt": wt, "bb": bb, "lw": lw,
        })
    res = run_bass_kernel_spmd(nc, in_maps, core_ids=list(range(N_CORES)))
    LAST_RESULTS = res
    if res.exec_time_ns is not None:
        print(f"HW exec time: {res.exec_time_ns} ns")
    out = np.concatenate([res.results[c]["out"] for c in range(N_CORES)], axis=0)
    return np.ascontiguousarray(out.astype(np.float32))
